# revision 78
# baseline (speedup 1.0000x reference)
"""Trainium2 Bass kernel for LorentzInvariantPositionalEncoding.

Reference computation (B=32, N=512, D=512):
  out[b,i,d] = x[b,i,d] + pe[i,d]
  arg[b,i,j] = sum_{k=1..3} (xc[b,i,k]-xc[b,j,k])^2 - (xc[b,i,0]-xc[b,j,0])^2
  ld[b,i,j]  = sqrt(relu(arg))        (== reference's masked sqrt)

Strategy: pure data parallel over batch, 4 batches per core on 8 cores.
This problem is HBM-bound, so the large tensors (x, pe, out, ld) travel as
bf16 (host converts; the harness tolerance is 2e-2 and bf16 quantization
costs ~2e-3 scale-relative). x_coords stays f32: the Minkowski Gram matmul
  arg = q_i + q_j - 2 * <c_i, eta*c_j>,   q_i = sum_k eta_k c_ik^2
cancels catastrophically near the light cone, so operands keep the
Dekker-style hi/lo f32r split (K=16 matmul; f32r streams 1 cycle/row and
matmul cost is independent of K).  ld is symmetric, so only the block upper
triangle (j >= i at 128-granularity) is computed and stored; the host
mirrors the lower blocks while widening bf16 -> f32.

Schedule: the lorentz chain gates the bulk of the store traffic, so xc
loads first and alone on the low-latency HWDGE sync ring (its completion
receipt gates everything); the identity for the PE transposes is built
on-chip via affine_select (no DMA to wait for); x and pe ride the SWDGE
gpsimd ring (latency-tolerant bulk).  Coord assembly is eta-free (no const
dependency) and batched across all 4 batches in one set of wide DVE ops;
am/bm/identity are f32r so every assembly write rounds (what the Dekker
split wants) and the PE streams them 4x faster.  am/bm columns are ordered
(g-group, batch, 32-slot) so ONE [128,128] transpose per g-group lands all
4 batches' K-rows at psum partitions {0,32,64,96} -- legal quadrant bases
for matmul operands -- and ONE strided DVE copy un-permutes the column
order (j = 4p+g -> true order, required by the triangle chunking) for all
batches and both operands while evacuating psum.  Per batch the 4 triangle
matmuls pack one 3-bank psum tile; relu is folded into the sqrt: ACT sqrt
reads the psum arg directly (negatives -> NaN), writes bf16, and a 4x-mode
DVE tensor_scalar_max(., 0) zeroes the NaNs (HW-verified maxNum
semantics).  The x+pe adds (DVE 2x bf16) are emitted last -- lowest
scheduler preference, so a ready max always wins the DVE -- but released
early in the scheduler's simulated clock so they fill the post-cast DVE
gap; ld stores spread across the sync and gpsimd rings.
"""

from contextlib import ExitStack

import numpy as np
import ml_dtypes

import concourse.bass as bass
import concourse.tile as tile
from concourse import bacc, mybir
from concourse.bass_utils import run_bass_kernel_spmd

B, N, D = 32, 512, 512
MAX_LEN = 5000
NCORES = 8
BP = B // NCORES  # batches per core
P = 128
NCH = N // P  # 4 partition chunks of the i dimension
K = 16

_F32 = mybir.dt.float32
_F32R = mybir.dt.float32r
_BF16 = mybir.dt.bfloat16
_NPBF16 = ml_dtypes.bfloat16

# psum/ldt packing offsets for the 4 upper-triangle chunks of one batch:
# all four chunks pack into one 3-bank psum tile (a matmul output must not
# cross a 2KB psum bank, so chunk 2 starts at the bank-2 boundary, leaving
# a 128-column hole that the sqrt fills with garbage and nothing stores).
_CHUNK_COLS = [N - P * n for n in range(NCH)]  # 512, 384, 256, 128
_PSUM_OFF = [0, 512, 1024, 1280]
_LDT_OFF = _PSUM_OFF
_LDT_LEN = 1408

_cached_nc = None


def _build():
    global _cached_nc
    if _cached_nc is not None:
        return _cached_nc

    nc = bacc.Bacc("TRN2", target_bir_lowering=False, debug=False, num_devices=NCORES)

    x_in = nc.dram_tensor("x", [BP, N, D], _BF16, kind="ExternalInput")
    xc_in = nc.dram_tensor("xc", [BP, N, 4], _F32, kind="ExternalInput")
    pe_in = nc.dram_tensor("pe", [N, D], _BF16, kind="ExternalInput")
    out_o = nc.dram_tensor("out", [BP, N, D], _BF16, kind="ExternalOutput")
    ld_o = nc.dram_tensor("ld", [BP, N, N], _BF16, kind="ExternalOutput")

    G = BP * NCH  # 16 (batch, row-group) pairs

    with tile.TileContext(nc) as tc, ExitStack() as ctx:
        cpool = ctx.enter_context(tc.tile_pool(name="const", bufs=1))
        xpool = ctx.enter_context(tc.tile_pool(name="x", bufs=4))
        ldpool = ctx.enter_context(tc.tile_pool(name="ld", bufs=4))
        mpool = ctx.enter_context(tc.tile_pool(name="mats", bufs=3))
        parg = ctx.enter_context(tc.tile_pool(name="parg", bufs=2, space="PSUM"))
        ptp = ctx.enter_context(tc.tile_pool(name="ptp", bufs=1, space="PSUM"))

        # force the sqrt table set resident during the preamble, off the
        # critical path of the first real sqrt
        dum = cpool.tile([1, 8], _F32)
        nc.vector.memset(dum[:], 4.0)
        nc.scalar.sqrt(dum[:], dum[:])

        # coords first on the low-latency HWDGE ring: they gate everything.
        # Partition p holds rows 4p+q (q=0..3) of each batch -- 64B runs.
        ct_all = cpool.tile([P, G * 4], _F32)
        nc.sync.dma_start(
            ct_all[:].rearrange("p (b q k) -> p b q k", b=BP, q=NCH),
            xc_in.rearrange("b (p q) k -> p b q k", q=NCH),
        )
        # identity built on-chip (no DMA to wait for): affine_select keeps
        # in_ where the per-element iota f - p == 0.  f32r so the transpose
        # streams it at 1 cycle/col, and so the f32r-producer check passes.
        ones_t = cpool.tile([P, P], _F32R)
        nc.gpsimd.memset(ones_t[:].bitcast(_F32), 1.0)
        ident_t = cpool.tile([P, P], _F32R)
        nc.gpsimd.affine_select(
            ident_t[:],
            ones_t[:],
            pattern=[[1, P]],
            compare_op=mybir.AluOpType.is_equal,
            fill=0.0,
            base=0,
            channel_multiplier=-1,
        )
        ident = ident_t[:]
        # tiny dummy matmul absorbs the PE first-op warm-up (~0.6us of
        # extra latency on the first real transpose/matmul) off the
        # critical path, using the identity as both (f32r) operands and
        # scribbling into the transpose psum tile (overwritten later)
        tp = ptp.tile([P, 2 * N], _F32R, tag="tp")
        nc.tensor.matmul(
            tp[0:16, 0:8].bitcast(_F32), ident_t[0:16, 0:16],
            ident_t[0:16, 0:8], start=True, stop=True,
        )

        # x and pe loads on the SWDGE ring: bulk and latency-tolerant, and
        # keeping the HWDGE rings empty lets xc's completion receipt (which
        # gates the whole lorentz chain) come back fast
        xts = []
        for b in range(BP):
            xt = xpool.tile([P, NCH * D], _BF16)
            nc.gpsimd.dma_start(
                xt[:].rearrange("p (n d) -> p n d", n=NCH),
                x_in[b].rearrange("(n p) d -> p n d", p=P),
            )
            xts.append(xt)
        pe_t = cpool.tile([P, NCH * D], _BF16)
        nc.gpsimd.dma_start(
            pe_t[:].rearrange("p (n d) -> p n d", n=NCH),
            pe_in.rearrange("(n p) d -> p n d", p=P),
        )

        # ---- operand assembly ----
        # eta-free forms (no const dependency): q = c1^2+c2^2+c3^2-c0^2,
        # and the -2*eta scaling is tensor_scalar ops with a k=0 sign fixup.
        # Row pairing (lhsT row, rhs row) by k:
        #  k 0-3: (-2e*ch, ch)  4-7: (-2e*ch, cl)  8-11: (-2e*cl, ch)
        #  k 12: (qh, 1)  13: (ql, 1)  14: (1, qh)  15: (1, ql)
        # am/bm are f32r tiles: every write rounds (the Dekker split wants
        # that), and f32r weights stream 4x faster through the PE.
        # Columns are (g, batch, 32-slot) so that ONE [128,128] transpose
        # per g-group lands all 4 batches' K-rows at psum partitions
        # {0,32,64,96} -- each a legal quadrant base for the matmuls.
        C = 2 * K  # 32-slot groups (16 real + 16 pad)
        t2 = cpool.tile([P, G * 4], _F32)
        q_pp = cpool.tile([P, G], _F32)
        v_pp = cpool.tile([P, G], _F32)
        w_pp = cpool.tile([P, G], _F32)
        am = cpool.tile([P, NCH * BP * C], _F32R)
        bm = cpool.tile([P, NCH * BP * C], _F32R)

        ct4 = ct_all[:].rearrange("p (b g k) -> p g b k", b=BP, g=NCH)
        q4 = q_pp[:].rearrange("p (b g u) -> p g b u", b=BP, g=NCH, u=1)
        nc.vector.tensor_mul(t2[:], ct_all[:], ct_all[:])
        v3 = v_pp[:].rearrange("p (g u) -> p g u", u=1)
        t23 = t2[:].rearrange("p (g k) -> p g k", g=G)
        nc.vector.tensor_add(v3, t23[:, :, 1:2], t23[:, :, 2:3])
        w3 = w_pp[:].rearrange("p (g u) -> p g u", u=1)
        nc.vector.tensor_sub(w3, t23[:, :, 3:4], t23[:, :, 0:1])
        nc.vector.tensor_add(q_pp[:], v_pp[:], w_pp[:])

        a4 = am[:].rearrange("p (g b c) -> p g b c", g=NCH, b=BP)
        nc.vector.tensor_copy(a4[:, :, :, 0:4], ct4)  # ch
        nc.vector.tensor_sub(a4[:, :, :, 4:8], ct4, a4[:, :, :, 0:4])  # cl
        nc.scalar.copy(a4[:, :, :, 8:12], a4[:, :, :, 0:4])
        nc.vector.memset(a4[:, :, :, 12:14].bitcast(_F32), 1.0)
        nc.vector.tensor_copy(a4[:, :, :, 14:15], q4)  # qh
        nc.vector.tensor_sub(a4[:, :, :, 15:16], q4, a4[:, :, :, 14:15])  # ql

        b4 = bm[:].rearrange("p (g b c) -> p g b c", g=NCH, b=BP)
        nc.vector.tensor_scalar_mul(b4[:, :, :, 0:4], a4[:, :, :, 0:4], -2.0)
        nc.vector.tensor_scalar_mul(b4[:, :, :, 0:1], a4[:, :, :, 0:1], 2.0)
        nc.scalar.copy(b4[:, :, :, 4:8], b4[:, :, :, 0:4])
        nc.vector.tensor_scalar_mul(b4[:, :, :, 8:12], a4[:, :, :, 4:8], -2.0)
        nc.vector.tensor_scalar_mul(b4[:, :, :, 8:9], a4[:, :, :, 4:5], 2.0)
        nc.scalar.copy(b4[:, :, :, 12:14], a4[:, :, :, 14:16])  # qh, ql
        nc.vector.memset(b4[:, :, :, 14:16].bitcast(_F32), 1.0)

        def emit_assemble():
            # 8 PE transposes land every batch's operands in one [128, 2N]
            # psum tile (rhs in the first N columns, lhsT in the last N;
            # batch b's K rows at partitions 32b).  ONE strided DVE copy
            # un-permutes columns (j = 4p+g order -> true order) for all
            # batches and both operands while evacuating psum: DVE op cost
            # is free-dim-driven, so the extra partitions are free.
            # all rhs (am) transposes first: the rhs half-cast that gates
            # every matmul can then start before the lhsT transposes finish
            for g in range(NCH):
                nc.tensor.transpose(
                    tp[:, g * P : (g + 1) * P], am[:, g * P : (g + 1) * P], ident
                )
            for g in range(NCH):
                nc.tensor.transpose(
                    tp[:, N + g * P : N + (g + 1) * P],
                    bm[:, g * P : (g + 1) * P],
                    ident,
                )
            ops = mpool.tile([P, 2 * N], _F32R, tag="ops")
            nc.vector.tensor_copy(
                ops[:, 0:N].rearrange("k (p g) -> k g p", g=NCH), tp[:, 0:N]
            )
            nc.vector.tensor_copy(
                ops[:, N : 2 * N].rearrange("k (p g) -> k g p", g=NCH),
                tp[:, N : 2 * N],
            )
            return ops

        def emit_lorentz(b, ops):
            # block upper triangle only: chunk n = rows [128n,128n+128),
            # cols [128n, 512).  chunks 0,1 share a 2-bank psum tile and one
            # sqrt; chunks 2,3 a 1-bank tile.
            boff = 32 * b
            ldt = ldpool.tile([P, _LDT_LEN], _BF16)
            argp = parg.tile([P, _LDT_LEN], _F32, tag="arg")
            for n in range(NCH):
                nc.tensor.matmul(
                    argp[:, _PSUM_OFF[n] : _PSUM_OFF[n] + _CHUNK_COLS[n]],
                    ops[boff : boff + K, N + n * P : N + (n + 1) * P],
                    ops[boff : boff + K, n * P : N],
                    start=True,
                    stop=True,
                    tile_position=(boff, 0),
                )
            # ONE sqrt straight off psum (negatives -> NaN), bf16 out, then
            # ONE maxNum(NaN|neg, 0) = 0 in 4x-mode bf16
            nc.scalar.sqrt(ldt[:], argp[:])
            # delay the max in SIM time only: the scheduler's clock runs
            # fast and never sees the real 3us DVE idle before sqrt(0)
            # completes, so without this the adds are ordered after the
            # maxes and that gap goes unused
            with tc.tile_wait_until(0.012 + 0.001 * b):
                nc.vector.tensor_scalar_max(ldt[:], ldt[:], 0.0)
            # spread store issues: 16 issues on one engine would
            # serialize the tail; scalar stays free for sqrts
            for n in range(NCH):
                seng = nc.sync if n < 2 else nc.gpsimd
                seng.dma_start(
                    ld_o[b, n * P : (n + 1) * P, n * P : N],
                    ldt[:, _LDT_OFF[n] : _LDT_OFF[n] + _CHUNK_COLS[n]],
                )

        def emit_add(b):
            # DVE 2x-mode bf16 add; tile_wait_until pushes it late in the
            # scheduler's simulated clock so it fills gaps BEHIND the
            # lorentz-chain casts/maxes instead of delaying them
            xt = xts[b]
            with tc.tile_wait_until(0.004 + 0.0003 * b):
                nc.vector.tensor_add(xt[:], xt[:], pe_t[:])
            nc.sync.dma_start(
                out_o[b].rearrange("(n p) d -> p n d", p=P),
                xt[:].rearrange("p (n d) -> p n d", n=NCH),
            )

        # lorentz chain leads; adds emitted last (lowest scheduler
        # preference, so a ready max always wins the DVE) but released
        # early in sim time so they fill the post-cast gap
        ops = emit_assemble()
        for b in range(BP):
            emit_lorentz(b, ops)
        for b in range(BP):
            emit_add(b)

    nc.finalize()
    _cached_nc = nc
    return nc


def _run(x, x_coords, pe, trace=False):
    x = np.asarray(x)
    x_coords = np.ascontiguousarray(np.asarray(x_coords), dtype=np.float32)
    pe = np.asarray(pe)
    assert x.shape == (B, N, D) and x_coords.shape == (B, N, 4)
    assert pe.shape == (MAX_LEN, D)
    xb = np.ascontiguousarray(x).astype(_NPBF16)
    peb = np.ascontiguousarray(pe[0:N]).astype(_NPBF16)

    nc = _build()
    in_maps = [
        {
            "x": xb[i * BP : (i + 1) * BP],
            "xc": x_coords[i * BP : (i + 1) * BP],
            "pe": peb,
        }
        for i in range(NCORES)
    ]
    res = run_bass_kernel_spmd(nc, in_maps, list(range(NCORES)), trace=trace)
    out = np.concatenate(
        [res.results[i]["out"] for i in range(NCORES)], axis=0
    ).astype(np.float32)
    ld = np.concatenate(
        [res.results[i]["ld"] for i in range(NCORES)], axis=0
    ).astype(np.float32)
    # mirror the block lower triangle from the stored upper blocks
    for bn in range(1, NCH):
        rn = slice(bn * P, (bn + 1) * P)
        for bm in range(bn):
            rm = slice(bm * P, (bm + 1) * P)
            ld[:, rn, rm] = ld[:, rm, rn].transpose(0, 2, 1)
    return (out, ld), res


def kernel(x, x_coords, pe):
    (out, ld), _ = _run(x, x_coords, pe, trace=False)
    return (out, ld)


# revision 79
# speedup vs baseline: 1.0605x; 1.0605x over previous
"""Trainium2 Bass kernel for LorentzInvariantPositionalEncoding.

Reference computation (B=32, N=512, D=512):
  out[b,i,d] = x[b,i,d] + pe[i,d]
  arg[b,i,j] = sum_{k=1..3} (xc[b,i,k]-xc[b,j,k])^2 - (xc[b,i,0]-xc[b,j,0])^2
  ld[b,i,j]  = sqrt(relu(arg))        (== reference's masked sqrt)

Strategy: pure data parallel over batch, 4 batches per core on 8 cores.
This problem is HBM-bound, so the large tensors (x, pe, out, ld) travel as
bf16 (host converts; the harness tolerance is 2e-2 and bf16 quantization
costs ~2e-3 scale-relative). x_coords stays f32: the Minkowski Gram matmul
  arg = q_i + q_j - 2 * <c_i, eta*c_j>,   q_i = sum_k eta_k c_ik^2
cancels catastrophically near the light cone, so operands keep the
Dekker-style hi/lo f32r split (K=16 matmul; f32r streams 1 cycle/row and
matmul cost is independent of K).  ld is symmetric, so only the block upper
triangle (j >= i at 128-granularity) is computed and stored; the host
mirrors the lower blocks while widening bf16 -> f32.

Schedule: the lorentz chain gates the bulk of the store traffic, so xc
loads first and alone on the low-latency HWDGE sync ring (its completion
receipt gates everything); the identity for the PE transposes is built
on-chip via affine_select (no DMA to wait for); x and pe ride the SWDGE
gpsimd ring (latency-tolerant bulk).  Coord assembly is eta-free (no const
dependency) and batched across all 4 batches in one set of wide DVE ops;
am/bm/identity are f32r so every assembly write rounds (what the Dekker
split wants) and the PE streams them 4x faster.  am/bm columns are ordered
(g-group, batch, 32-slot) so ONE [128,128] transpose per g-group lands all
4 batches' K-rows at psum partitions {0,32,64,96} -- legal quadrant bases
for matmul operands -- and ONE strided DVE copy un-permutes the column
order (j = 4p+g -> true order, required by the triangle chunking) for all
batches and both operands while evacuating psum.  Per batch the 4 triangle
matmuls pack one 3-bank psum tile; relu is folded into the sqrt: ACT sqrt
reads the psum arg directly (negatives -> NaN), writes bf16, and a 4x-mode
DVE tensor_scalar_max(., 0) zeroes the NaNs (HW-verified maxNum
semantics).  The x+pe adds (DVE 2x bf16) are emitted last -- lowest
scheduler preference, so a ready max always wins the DVE -- but released
early in the scheduler's simulated clock so they fill the post-cast DVE
gap; ld stores spread across the sync and gpsimd rings.
"""

from contextlib import ExitStack

import numpy as np
import ml_dtypes

import concourse.bass as bass
import concourse.tile as tile
from concourse import bacc, mybir
from concourse.bass_utils import run_bass_kernel_spmd

B, N, D = 32, 512, 512
MAX_LEN = 5000
NCORES = 8
BP = B // NCORES  # batches per core
P = 128
NCH = N // P  # 4 partition chunks of the i dimension
K = 16

_F32 = mybir.dt.float32
_F32R = mybir.dt.float32r
_BF16 = mybir.dt.bfloat16
_NPBF16 = ml_dtypes.bfloat16

# psum/ldt packing offsets for the 4 upper-triangle chunks of one batch:
# all four chunks pack into one 3-bank psum tile (a matmul output must not
# cross a 2KB psum bank, so chunk 2 starts at the bank-2 boundary, leaving
# a 128-column hole that the sqrt fills with garbage and nothing stores).
_CHUNK_COLS = [N - P * n for n in range(NCH)]  # 512, 384, 256, 128
_PSUM_OFF = [0, 512, 1024, 1280]
_LDT_OFF = _PSUM_OFF
_LDT_LEN = 1408

_cached_nc = None


def _build():
    global _cached_nc
    if _cached_nc is not None:
        return _cached_nc

    nc = bacc.Bacc("TRN2", target_bir_lowering=False, debug=False, num_devices=NCORES)

    x_in = nc.dram_tensor("x", [BP, N, D], _BF16, kind="ExternalInput")
    xc_in = nc.dram_tensor("xc", [BP, N, 4], _F32, kind="ExternalInput")
    pe_in = nc.dram_tensor("pe", [N, D], _BF16, kind="ExternalInput")
    out_o = nc.dram_tensor("out", [BP, N, D], _BF16, kind="ExternalOutput")
    ld_o = nc.dram_tensor("ld", [BP, N, N], _BF16, kind="ExternalOutput")

    G = BP * NCH  # 16 (batch, row-group) pairs

    with tile.TileContext(nc) as tc, ExitStack() as ctx:
        cpool = ctx.enter_context(tc.tile_pool(name="const", bufs=1))
        xpool = ctx.enter_context(tc.tile_pool(name="x", bufs=4))
        ldpool = ctx.enter_context(tc.tile_pool(name="ld", bufs=4))
        mpool = ctx.enter_context(tc.tile_pool(name="mats", bufs=3))
        parg = ctx.enter_context(tc.tile_pool(name="parg", bufs=2, space="PSUM"))
        ptp = ctx.enter_context(tc.tile_pool(name="ptp", bufs=1, space="PSUM"))

        # force the sqrt table set resident during the preamble, off the
        # critical path of the first real sqrt
        dum = cpool.tile([1, 8], _F32)
        nc.vector.memset(dum[:], 4.0)
        nc.scalar.sqrt(dum[:], dum[:])

        # coords first on the low-latency HWDGE ring: they gate everything.
        # Partition p holds rows 4p+q (q=0..3) of each batch -- 64B runs.
        ct_all = cpool.tile([P, G * 4], _F32)
        nc.sync.dma_start(
            ct_all[:].rearrange("p (b q k) -> p b q k", b=BP, q=NCH),
            xc_in.rearrange("b (p q) k -> p b q k", q=NCH),
        )
        # identity built on-chip (no DMA to wait for): affine_select keeps
        # in_ where the per-element iota f - p == 0.  f32r so the transpose
        # streams it at 1 cycle/col, and so the f32r-producer check passes.
        ones_t = cpool.tile([P, P], _F32R)
        nc.gpsimd.memset(ones_t[:].bitcast(_F32), 1.0)
        ident_t = cpool.tile([P, P], _F32R)
        nc.gpsimd.affine_select(
            ident_t[:],
            ones_t[:],
            pattern=[[1, P]],
            compare_op=mybir.AluOpType.is_equal,
            fill=0.0,
            base=0,
            channel_multiplier=-1,
        )
        ident = ident_t[:]
        # tiny dummy matmul absorbs the PE first-op warm-up (~0.6us of
        # extra latency on the first real transpose/matmul) off the
        # critical path, using the identity as both (f32r) operands and
        # scribbling into the transpose psum tile (overwritten later)
        tp = ptp.tile([P, 2 * N], _F32R, tag="tp")
        nc.tensor.matmul(
            tp[0:16, 0:8].bitcast(_F32), ident_t[0:16, 0:16],
            ident_t[0:16, 0:8], start=True, stop=True,
        )

        # x and pe loads on the SWDGE ring: bulk and latency-tolerant, and
        # keeping the HWDGE rings empty lets xc's completion receipt (which
        # gates the whole lorentz chain) come back fast
        xts = []
        for b in range(BP):
            xt = xpool.tile([P, NCH * D], _BF16)
            nc.gpsimd.dma_start(
                xt[:].rearrange("p (n d) -> p n d", n=NCH),
                x_in[b].rearrange("(n p) d -> p n d", p=P),
            )
            xts.append(xt)
        pe_t = cpool.tile([P, NCH * D], _BF16)
        nc.gpsimd.dma_start(
            pe_t[:].rearrange("p (n d) -> p n d", n=NCH),
            pe_in.rearrange("(n p) d -> p n d", p=P),
        )

        # ---- operand assembly ----
        # eta-free forms (no const dependency): q = c1^2+c2^2+c3^2-c0^2,
        # and the -2*eta scaling is tensor_scalar ops with a k=0 sign fixup.
        # Row pairing (lhsT row, rhs row) by k:
        #  k 0-3: (-2e*ch, ch)  4-7: (-2e*ch, cl)  8-11: (-2e*cl, ch)
        #  k 12: (qh, 1)  13: (ql, 1)  14: (1, qh)  15: (1, ql)
        # am/bm are f32r tiles: every write rounds (the Dekker split wants
        # that), and f32r weights stream 4x faster through the PE.
        # Columns are (g, batch, 32-slot) so that ONE [128,128] transpose
        # per g-group lands all 4 batches' K-rows at psum partitions
        # {0,32,64,96} -- each a legal quadrant base for the matmuls.
        C = 2 * K  # 32-slot groups (16 real + 16 pad)
        t2 = cpool.tile([P, G * 4], _F32)
        q_pp = cpool.tile([P, G], _F32)
        v_pp = cpool.tile([P, G], _F32)
        w_pp = cpool.tile([P, G], _F32)
        am = cpool.tile([P, NCH * BP * C], _F32R)
        bm = cpool.tile([P, NCH * BP * C], _F32R)

        ct4 = ct_all[:].rearrange("p (b g k) -> p g b k", b=BP, g=NCH)
        q4 = q_pp[:].rearrange("p (b g u) -> p g b u", b=BP, g=NCH, u=1)
        nc.vector.tensor_mul(t2[:], ct_all[:], ct_all[:])
        v3 = v_pp[:].rearrange("p (g u) -> p g u", u=1)
        t23 = t2[:].rearrange("p (g k) -> p g k", g=G)
        nc.vector.tensor_add(v3, t23[:, :, 1:2], t23[:, :, 2:3])
        w3 = w_pp[:].rearrange("p (g u) -> p g u", u=1)
        nc.vector.tensor_sub(w3, t23[:, :, 3:4], t23[:, :, 0:1])
        nc.vector.tensor_add(q_pp[:], v_pp[:], w_pp[:])

        a4 = am[:].rearrange("p (g b c) -> p g b c", g=NCH, b=BP)
        nc.vector.tensor_copy(a4[:, :, :, 0:4], ct4)  # ch
        nc.vector.tensor_sub(a4[:, :, :, 4:8], ct4, a4[:, :, :, 0:4])  # cl
        nc.scalar.copy(a4[:, :, :, 8:12], a4[:, :, :, 0:4])
        nc.vector.memset(a4[:, :, :, 12:14].bitcast(_F32), 1.0)
        nc.vector.tensor_copy(a4[:, :, :, 14:15], q4)  # qh
        nc.vector.tensor_sub(a4[:, :, :, 15:16], q4, a4[:, :, :, 14:15])  # ql

        b4 = bm[:].rearrange("p (g b c) -> p g b c", g=NCH, b=BP)
        nc.vector.tensor_scalar_mul(b4[:, :, :, 0:4], a4[:, :, :, 0:4], -2.0)
        nc.vector.tensor_scalar_mul(b4[:, :, :, 0:1], a4[:, :, :, 0:1], 2.0)
        nc.scalar.copy(b4[:, :, :, 4:8], b4[:, :, :, 0:4])
        nc.vector.tensor_scalar_mul(b4[:, :, :, 8:12], a4[:, :, :, 4:8], -2.0)
        nc.vector.tensor_scalar_mul(b4[:, :, :, 8:9], a4[:, :, :, 4:5], 2.0)
        nc.scalar.copy(b4[:, :, :, 12:14], a4[:, :, :, 14:16])  # qh, ql
        nc.vector.memset(b4[:, :, :, 14:16].bitcast(_F32), 1.0)

        def emit_assemble():
            # 8 PE transposes land every batch's operands in one [128, 2N]
            # psum tile (rhs in the first N columns, lhsT in the last N;
            # batch b's K rows at partitions 32b).  ONE strided DVE copy
            # un-permutes columns (j = 4p+g order -> true order) for all
            # batches and both operands while evacuating psum: DVE op cost
            # is free-dim-driven, so the extra partitions are free.
            # all rhs (am) transposes first: the rhs half-cast that gates
            # every matmul can then start before the lhsT transposes finish
            for g in range(NCH):
                nc.tensor.transpose(
                    tp[:, g * P : (g + 1) * P], am[:, g * P : (g + 1) * P], ident
                )
            for g in range(NCH):
                nc.tensor.transpose(
                    tp[:, N + g * P : N + (g + 1) * P],
                    bm[:, g * P : (g + 1) * P],
                    ident,
                )
            ops = mpool.tile([P, 2 * N], _F32R, tag="ops")
            nc.vector.tensor_copy(
                ops[:, 0:N].rearrange("k (p g) -> k g p", g=NCH), tp[:, 0:N]
            )
            nc.vector.tensor_copy(
                ops[:, N : 2 * N].rearrange("k (p g) -> k g p", g=NCH),
                tp[:, N : 2 * N],
            )
            return ops

        def emit_lorentz(b, ops):
            # block upper triangle only: chunk n = rows [128n,128n+128),
            # cols [128n, 512).  chunks 0,1 share a 2-bank psum tile and one
            # sqrt; chunks 2,3 a 1-bank tile.
            boff = 32 * b
            ldt = ldpool.tile([P, _LDT_LEN], _BF16)
            argp = parg.tile([P, _LDT_LEN], _F32, tag="arg")
            for n in range(NCH):
                nc.tensor.matmul(
                    argp[:, _PSUM_OFF[n] : _PSUM_OFF[n] + _CHUNK_COLS[n]],
                    ops[boff : boff + K, N + n * P : N + (n + 1) * P],
                    ops[boff : boff + K, n * P : N],
                    start=True,
                    stop=True,
                    tile_position=(boff, 0),
                )
            # ONE sqrt straight off psum (negatives -> NaN), bf16 out, then
            # ONE maxNum(NaN|neg, 0) = 0 in 4x-mode bf16
            nc.scalar.sqrt(ldt[:], argp[:])
            nc.vector.tensor_scalar_max(ldt[:], ldt[:], 0.0)
            # spread store issues: 16 issues on one engine would
            # serialize the tail; scalar stays free for sqrts
            for n in range(NCH):
                seng = nc.sync if n < 2 else nc.gpsimd
                seng.dma_start(
                    ld_o[b, n * P : (n + 1) * P, n * P : N],
                    ldt[:, _LDT_OFF[n] : _LDT_OFF[n] + _CHUNK_COLS[n]],
                )

        def emit_add(b):
            # DVE 2x-mode bf16 add; tile_wait_until pushes it late in the
            # scheduler's simulated clock so it fills gaps BEHIND the
            # lorentz-chain casts/maxes instead of delaying them
            xt = xts[b]
            with tc.tile_wait_until(0.004 + 0.0003 * b):
                nc.vector.tensor_add(xt[:], xt[:], pe_t[:])
            nc.sync.dma_start(
                out_o[b].rearrange("(n p) d -> p n d", p=P),
                xt[:].rearrange("p (n d) -> p n d", n=NCH),
            )

        # lorentz chain leads; adds emitted last (lowest scheduler
        # preference, so a ready max always wins the DVE) but released
        # early in sim time so they fill the post-cast gap
        ops = emit_assemble()
        for b in range(BP):
            emit_lorentz(b, ops)
        for b in range(BP):
            emit_add(b)

    nc.finalize()
    _cached_nc = nc
    return nc


def _run(x, x_coords, pe, trace=False):
    x = np.asarray(x)
    x_coords = np.ascontiguousarray(np.asarray(x_coords), dtype=np.float32)
    pe = np.asarray(pe)
    assert x.shape == (B, N, D) and x_coords.shape == (B, N, 4)
    assert pe.shape == (MAX_LEN, D)
    xb = np.ascontiguousarray(x).astype(_NPBF16)
    peb = np.ascontiguousarray(pe[0:N]).astype(_NPBF16)

    nc = _build()
    in_maps = [
        {
            "x": xb[i * BP : (i + 1) * BP],
            "xc": x_coords[i * BP : (i + 1) * BP],
            "pe": peb,
        }
        for i in range(NCORES)
    ]
    res = run_bass_kernel_spmd(nc, in_maps, list(range(NCORES)), trace=trace)
    out = np.concatenate(
        [res.results[i]["out"] for i in range(NCORES)], axis=0
    ).astype(np.float32)
    ld = np.concatenate(
        [res.results[i]["ld"] for i in range(NCORES)], axis=0
    ).astype(np.float32)
    # mirror the block lower triangle from the stored upper blocks
    for bn in range(1, NCH):
        rn = slice(bn * P, (bn + 1) * P)
        for bm in range(bn):
            rm = slice(bm * P, (bm + 1) * P)
            ld[:, rn, rm] = ld[:, rm, rn].transpose(0, 2, 1)
    return (out, ld), res


def kernel(x, x_coords, pe):
    (out, ld), _ = _run(x, x_coords, pe, trace=False)
    return (out, ld)


# revision 80
# speedup vs baseline: 1.1397x; 1.0747x over previous
"""Trainium2 Bass kernel for LorentzInvariantPositionalEncoding.

Reference computation (B=32, N=512, D=512):
  out[b,i,d] = x[b,i,d] + pe[i,d]
  arg[b,i,j] = sum_{k=1..3} (xc[b,i,k]-xc[b,j,k])^2 - (xc[b,i,0]-xc[b,j,0])^2
  ld[b,i,j]  = sqrt(relu(arg))        (== reference's masked sqrt)

Strategy: pure data parallel over batch, 4 batches per core on 8 cores.
This problem is HBM-bound, so the large tensors (x, pe, out, ld) travel as
bf16 (host converts; the harness tolerance is 2e-2 and bf16 quantization
costs ~2e-3 scale-relative). x_coords stays f32: the Minkowski Gram matmul
  arg = q_i + q_j - 2 * <c_i, eta*c_j>,   q_i = sum_k eta_k c_ik^2
cancels catastrophically near the light cone, so operands keep the
Dekker-style hi/lo f32r split (K=16 matmul; f32r streams 1 cycle/row and
matmul cost is independent of K).  ld is symmetric, so only the block upper
triangle (j >= i at 128-granularity) is computed and stored; the host
mirrors the lower blocks while widening bf16 -> f32.

Schedule: the lorentz chain gates the bulk of the store traffic, so xc
loads first and alone on the low-latency HWDGE sync ring (its completion
receipt gates everything); the identity for the PE transposes is built
on-chip via affine_select (no DMA to wait for); x and pe ride the SWDGE
gpsimd ring (latency-tolerant bulk).  Coord assembly is eta-free (no const
dependency) and batched across all 4 batches in one set of wide DVE ops;
am/bm/identity are f32r so every assembly write rounds (what the Dekker
split wants) and the PE streams them 4x faster.  am/bm columns are ordered
(g-group, batch, 32-slot) so ONE [128,128] transpose per g-group lands all
4 batches' K-rows at psum partitions {0,32,64,96} -- legal quadrant bases
for matmul operands -- and ONE strided DVE copy un-permutes the column
order (j = 4p+g -> true order, required by the triangle chunking) for all
batches and both operands while evacuating psum.  Per batch the 4 triangle
matmuls pack one 3-bank psum tile; relu is folded into the sqrt: ACT sqrt
reads the psum arg directly (negatives -> NaN), writes bf16, and a 4x-mode
DVE tensor_scalar_max(., 0) zeroes the NaNs (HW-verified maxNum
semantics).  The x+pe adds (DVE 2x bf16) are emitted last -- lowest
scheduler preference, so a ready max always wins the DVE -- but released
early in the scheduler's simulated clock so they fill the post-cast DVE
gap; ld stores spread across the sync and gpsimd rings.
"""

from contextlib import ExitStack

import numpy as np
import ml_dtypes

import concourse.bass as bass
import concourse.tile as tile
from concourse import bacc, mybir
from concourse.bass_utils import run_bass_kernel_spmd

B, N, D = 32, 512, 512
MAX_LEN = 5000
NCORES = 8
BP = B // NCORES  # batches per core
P = 128
NCH = N // P  # 4 partition chunks of the i dimension
K = 16

_F32 = mybir.dt.float32
_F32R = mybir.dt.float32r
_BF16 = mybir.dt.bfloat16
_NPBF16 = ml_dtypes.bfloat16

# psum/ldt packing offsets for the 4 upper-triangle chunks of one batch:
# all four chunks pack into one 3-bank psum tile (a matmul output must not
# cross a 2KB psum bank, so chunk 2 starts at the bank-2 boundary, leaving
# a 128-column hole that the sqrt fills with garbage and nothing stores).
_CHUNK_COLS = [N - P * n for n in range(NCH)]  # 512, 384, 256, 128
_PSUM_OFF = [0, 512, 1024, 1280]
_LDT_OFF = _PSUM_OFF
_LDT_LEN = 1408

_cached_nc = None


def _build():
    global _cached_nc
    if _cached_nc is not None:
        return _cached_nc

    nc = bacc.Bacc("TRN2", target_bir_lowering=False, debug=False, num_devices=NCORES)

    x_in = nc.dram_tensor("x", [BP, N, D], _BF16, kind="ExternalInput")
    xc_in = nc.dram_tensor("xc", [BP, N, 4], _F32, kind="ExternalInput")
    pe_in = nc.dram_tensor("pe", [N, D], _BF16, kind="ExternalInput")
    out_o = nc.dram_tensor("out", [BP, N, D], _BF16, kind="ExternalOutput")
    ld_o = nc.dram_tensor("ld", [BP, N, N], _BF16, kind="ExternalOutput")

    G = BP * NCH  # 16 (batch, row-group) pairs

    with tile.TileContext(nc) as tc, ExitStack() as ctx:
        cpool = ctx.enter_context(tc.tile_pool(name="const", bufs=1))
        xpool = ctx.enter_context(tc.tile_pool(name="x", bufs=4))
        ldpool = ctx.enter_context(tc.tile_pool(name="ld", bufs=4))
        mpool = ctx.enter_context(tc.tile_pool(name="mats", bufs=3))
        parg = ctx.enter_context(tc.tile_pool(name="parg", bufs=2, space="PSUM"))
        ptp = ctx.enter_context(tc.tile_pool(name="ptp", bufs=1, space="PSUM"))

        # force the sqrt table set resident during the preamble, off the
        # critical path of the first real sqrt
        dum = cpool.tile([1, 8], _F32)
        nc.vector.memset(dum[:], 4.0)
        nc.scalar.sqrt(dum[:], dum[:])

        # coords first on the low-latency HWDGE ring: they gate everything.
        # Partition p holds rows 4p+q (q=0..3) of each batch -- 64B runs.
        ct_all = cpool.tile([P, G * 4], _F32)
        nc.sync.dma_start(
            ct_all[:].rearrange("p (b q k) -> p b q k", b=BP, q=NCH),
            xc_in.rearrange("b (p q) k -> p b q k", q=NCH),
        )
        # identity built on-chip (no DMA to wait for): affine_select keeps
        # in_ where the per-element iota f - p == 0.  f32r so the transpose
        # streams it at 1 cycle/col, and so the f32r-producer check passes.
        ones_t = cpool.tile([P, P], _F32R)
        nc.gpsimd.memset(ones_t[:].bitcast(_F32), 1.0)
        ident_t = cpool.tile([P, P], _F32R)
        nc.gpsimd.affine_select(
            ident_t[:],
            ones_t[:],
            pattern=[[1, P]],
            compare_op=mybir.AluOpType.is_equal,
            fill=0.0,
            base=0,
            channel_multiplier=-1,
        )
        ident = ident_t[:]
        # tiny dummy matmul absorbs the PE first-op warm-up (~0.6us of
        # extra latency on the first real transpose/matmul) off the
        # critical path, using the identity as both (f32r) operands and
        # scribbling into the transpose psum tile (overwritten later)
        tp = ptp.tile([P, 2 * N], _F32R, tag="tp")
        nc.tensor.matmul(
            tp[0:16, 0:8].bitcast(_F32), ident_t[0:16, 0:16],
            ident_t[0:16, 0:8], start=True, stop=True,
        )

        # x and pe loads on the SWDGE ring: bulk and latency-tolerant, and
        # keeping the HWDGE rings empty lets xc's completion receipt (which
        # gates the whole lorentz chain) come back fast
        # pe FIRST on the gpsimd ring: the scheduler sim models queue
        # order, and a late-modeled pe makes it believe the x+pe adds are
        # never ready before the maxes -- leaving the real 3us DVE gap
        # before sqrt(0) completion unfillable in the static order
        pe_t = cpool.tile([P, NCH * D], _BF16)
        nc.gpsimd.dma_start(
            pe_t[:].rearrange("p (n d) -> p n d", n=NCH),
            pe_in.rearrange("(n p) d -> p n d", p=P),
        )
        xts = []
        for b in range(BP):
            xt = xpool.tile([P, NCH * D], _BF16)
            nc.gpsimd.dma_start(
                xt[:].rearrange("p (n d) -> p n d", n=NCH),
                x_in[b].rearrange("(n p) d -> p n d", p=P),
            )
            xts.append(xt)

        # ---- operand assembly ----
        # eta-free forms (no const dependency): q = c1^2+c2^2+c3^2-c0^2,
        # and the -2*eta scaling is tensor_scalar ops with a k=0 sign fixup.
        # Row pairing (lhsT row, rhs row) by k:
        #  k 0-3: (-2e*ch, ch)  4-7: (-2e*ch, cl)  8-11: (-2e*cl, ch)
        #  k 12: (qh, 1)  13: (ql, 1)  14: (1, qh)  15: (1, ql)
        # am/bm are f32r tiles: every write rounds (the Dekker split wants
        # that), and f32r weights stream 4x faster through the PE.
        # Columns are (g, batch, 32-slot) so that ONE [128,128] transpose
        # per g-group lands all 4 batches' K-rows at psum partitions
        # {0,32,64,96} -- each a legal quadrant base for the matmuls.
        C = 2 * K  # 32-slot groups (16 real + 16 pad)
        t2 = cpool.tile([P, G * 4], _F32)
        q_pp = cpool.tile([P, G], _F32)
        v_pp = cpool.tile([P, G], _F32)
        w_pp = cpool.tile([P, G], _F32)
        am = cpool.tile([P, NCH * BP * C], _F32R)
        bm = cpool.tile([P, NCH * BP * C], _F32R)

        ct4 = ct_all[:].rearrange("p (b g k) -> p g b k", b=BP, g=NCH)
        q4 = q_pp[:].rearrange("p (b g u) -> p g b u", b=BP, g=NCH, u=1)
        nc.vector.tensor_mul(t2[:], ct_all[:], ct_all[:])
        v3 = v_pp[:].rearrange("p (g u) -> p g u", u=1)
        t23 = t2[:].rearrange("p (g k) -> p g k", g=G)
        nc.vector.tensor_add(v3, t23[:, :, 1:2], t23[:, :, 2:3])
        w3 = w_pp[:].rearrange("p (g u) -> p g u", u=1)
        nc.vector.tensor_sub(w3, t23[:, :, 3:4], t23[:, :, 0:1])
        nc.vector.tensor_add(q_pp[:], v_pp[:], w_pp[:])

        a4 = am[:].rearrange("p (g b c) -> p g b c", g=NCH, b=BP)
        nc.vector.tensor_copy(a4[:, :, :, 0:4], ct4)  # ch
        nc.vector.tensor_sub(a4[:, :, :, 4:8], ct4, a4[:, :, :, 0:4])  # cl
        nc.scalar.copy(a4[:, :, :, 8:12], a4[:, :, :, 0:4])
        nc.vector.memset(a4[:, :, :, 12:14].bitcast(_F32), 1.0)
        nc.vector.tensor_copy(a4[:, :, :, 14:15], q4)  # qh
        nc.vector.tensor_sub(a4[:, :, :, 15:16], q4, a4[:, :, :, 14:15])  # ql

        b4 = bm[:].rearrange("p (g b c) -> p g b c", g=NCH, b=BP)
        nc.vector.tensor_scalar_mul(b4[:, :, :, 0:4], a4[:, :, :, 0:4], -2.0)
        nc.vector.tensor_scalar_mul(b4[:, :, :, 0:1], a4[:, :, :, 0:1], 2.0)
        nc.scalar.copy(b4[:, :, :, 4:8], b4[:, :, :, 0:4])
        nc.vector.tensor_scalar_mul(b4[:, :, :, 8:12], a4[:, :, :, 4:8], -2.0)
        nc.vector.tensor_scalar_mul(b4[:, :, :, 8:9], a4[:, :, :, 4:5], 2.0)
        nc.scalar.copy(b4[:, :, :, 12:14], a4[:, :, :, 14:16])  # qh, ql
        nc.vector.memset(b4[:, :, :, 14:16].bitcast(_F32), 1.0)

        def emit_assemble():
            # 8 PE transposes land every batch's operands in one [128, 2N]
            # psum tile (rhs in the first N columns, lhsT in the last N;
            # batch b's K rows at partitions 32b).  ONE strided DVE copy
            # un-permutes columns (j = 4p+g order -> true order) for all
            # batches and both operands while evacuating psum: DVE op cost
            # is free-dim-driven, so the extra partitions are free.
            # all rhs (am) transposes first: the rhs half-cast that gates
            # every matmul can then start before the lhsT transposes finish
            for g in range(NCH):
                nc.tensor.transpose(
                    tp[:, g * P : (g + 1) * P], am[:, g * P : (g + 1) * P], ident
                )
            for g in range(NCH):
                nc.tensor.transpose(
                    tp[:, N + g * P : N + (g + 1) * P],
                    bm[:, g * P : (g + 1) * P],
                    ident,
                )
            ops = mpool.tile([P, 2 * N], _F32R, tag="ops")
            nc.vector.tensor_copy(
                ops[:, 0:N].rearrange("k (p g) -> k g p", g=NCH), tp[:, 0:N]
            )
            nc.vector.tensor_copy(
                ops[:, N : 2 * N].rearrange("k (p g) -> k g p", g=NCH),
                tp[:, N : 2 * N],
            )
            return ops

        def emit_lorentz(b, ops):
            # block upper triangle only: chunk n = rows [128n,128n+128),
            # cols [128n, 512).  chunks 0,1 share a 2-bank psum tile and one
            # sqrt; chunks 2,3 a 1-bank tile.
            boff = 32 * b
            ldt = ldpool.tile([P, _LDT_LEN], _BF16)
            argp = parg.tile([P, _LDT_LEN], _F32, tag="arg")
            for n in range(NCH):
                nc.tensor.matmul(
                    argp[:, _PSUM_OFF[n] : _PSUM_OFF[n] + _CHUNK_COLS[n]],
                    ops[boff : boff + K, N + n * P : N + (n + 1) * P],
                    ops[boff : boff + K, n * P : N],
                    start=True,
                    stop=True,
                    tile_position=(boff, 0),
                )
            # ONE sqrt straight off psum (negatives -> NaN), bf16 out, then
            # ONE maxNum(NaN|neg, 0) = 0 in 4x-mode bf16
            nc.scalar.sqrt(ldt[:], argp[:])
            nc.vector.tensor_scalar_max(ldt[:], ldt[:], 0.0)
            # spread store issues: 16 issues on one engine would
            # serialize the tail; scalar stays free for sqrts
            for n in range(NCH):
                seng = nc.sync if n < 2 else nc.gpsimd
                seng.dma_start(
                    ld_o[b, n * P : (n + 1) * P, n * P : N],
                    ldt[:, _LDT_OFF[n] : _LDT_OFF[n] + _CHUNK_COLS[n]],
                )

        def emit_add(b):
            # DVE 2x-mode bf16 add; tile_wait_until pushes it late in the
            # scheduler's simulated clock so it fills gaps BEHIND the
            # lorentz-chain casts/maxes instead of delaying them
            xt = xts[b]
            with tc.tile_wait_until(0.004 + 0.0003 * b):
                nc.vector.tensor_add(xt[:], xt[:], pe_t[:])
            nc.sync.dma_start(
                out_o[b].rearrange("(n p) d -> p n d", p=P),
                xt[:].rearrange("p (n d) -> p n d", n=NCH),
            )

        # lorentz chain leads; adds emitted last (lowest scheduler
        # preference, so a ready max always wins the DVE) but released
        # early in sim time so they fill the post-cast gap
        ops = emit_assemble()
        for b in range(BP):
            emit_lorentz(b, ops)
        for b in range(BP):
            emit_add(b)

    nc.finalize()
    _cached_nc = nc
    return nc


def _run(x, x_coords, pe, trace=False):
    x = np.asarray(x)
    x_coords = np.ascontiguousarray(np.asarray(x_coords), dtype=np.float32)
    pe = np.asarray(pe)
    assert x.shape == (B, N, D) and x_coords.shape == (B, N, 4)
    assert pe.shape == (MAX_LEN, D)
    xb = np.ascontiguousarray(x).astype(_NPBF16)
    peb = np.ascontiguousarray(pe[0:N]).astype(_NPBF16)

    nc = _build()
    in_maps = [
        {
            "x": xb[i * BP : (i + 1) * BP],
            "xc": x_coords[i * BP : (i + 1) * BP],
            "pe": peb,
        }
        for i in range(NCORES)
    ]
    res = run_bass_kernel_spmd(nc, in_maps, list(range(NCORES)), trace=trace)
    out = np.concatenate(
        [res.results[i]["out"] for i in range(NCORES)], axis=0
    ).astype(np.float32)
    ld = np.concatenate(
        [res.results[i]["ld"] for i in range(NCORES)], axis=0
    ).astype(np.float32)
    # mirror the block lower triangle from the stored upper blocks
    for bn in range(1, NCH):
        rn = slice(bn * P, (bn + 1) * P)
        for bm in range(bn):
            rm = slice(bm * P, (bm + 1) * P)
            ld[:, rn, rm] = ld[:, rm, rn].transpose(0, 2, 1)
    return (out, ld), res


def kernel(x, x_coords, pe):
    (out, ld), _ = _run(x, x_coords, pe, trace=False)
    return (out, ld)
